# revision 8
# baseline (speedup 1.0000x reference)
"""MixHopNet (GCN powers {0,1,2}) Trainium2 kernel, 8-core SPMD.

Strategy: partition destination nodes across 8 cores (1-D graph
partitioning).  Each core owns its node block and all edges whose
destination lands in that block.  Per propagate, source-node features
are fetched with int16 dma_gather from 4 source banks (<=32768 rows
each), scaled by the per-edge GCN norm, and scatter-added into the
owned block via one-hot selection matmuls (edges sorted by dst tile).
h1 is exchanged between the two propagates with an AllGather.  The
three linear layers + relu + output projection run per node tile in a
transposed layout so no activation transposes are needed beyond one
PE-transpose per operand tile.
"""

import sys

sys.path.insert(0, "/opt/trn_rl_repo")

import numpy as np

C = 8          # cores
P = 128        # partitions / tile height
CHUNK = 1024   # gather-call size in edge slots (hw ring limit ~1.5k descs)
CH_SUB = CHUNK // P
MAX_BANK = 32768


def _bank_split(rows):
    nb = max(1, -(-rows // MAX_BANK))
    b = -(-rows // nb)
    return nb, b


def _prep_edges(sa, da, w, src_rows, n, nd, nt, c):
    """Group (+pad) edges per core into (bank, dst-tile) slot arrays.

    sa/da: int64 src/dst node ids (all edges incl self loops)
    w: f32 edge weights; src_rows: size of the gather-source row space
    (sa must already be mapped into that row space).
    Returns dict with per-core idx16/meta arrays and static schedule.
    """
    nb, bsz = _bank_split(src_rows)
    core = da // nd
    r = da - core * nd
    tile = r // P
    dstl = r - tile * P
    bank = sa // bsz
    idx_in_bank = sa - bank * bsz

    # group id per edge: (core, bank, tile)
    g = (core * nb + bank) * nt + tile
    n_groups = C * nb * nt
    counts = np.bincount(g, minlength=n_groups).reshape(C, nb, nt)
    S = -(-counts.max(axis=0) // P)          # [nb, nt] subtiles per group

    # region = per-bank run of groups; pad each region to CHUNK slots
    reg_sub = S.sum(axis=1)                          # subtiles per bank
    reg_slots = reg_sub * P
    reg_slots_pad = -(-reg_slots // CHUNK) * CHUNK
    reg_base = np.concatenate([[0], np.cumsum(reg_slots_pad)])[:-1]
    tot = int(reg_slots_pad.sum())

    # base slot of each (bank, tile) group
    g_base = np.zeros((nb, nt), np.int64)
    for b in range(nb):
        g_base[b] = reg_base[b] + np.concatenate([[0], np.cumsum(S[b] * P)])[:-1]

    # static subtile schedule: (bank, tile) per subtile slot index
    sub_j = []          # dst tile per subtile (pad subtiles -> 0)
    for b in range(nb):
        for j in range(nt):
            sub_j += [j] * int(S[b, j])
        sub_j += [0] * int((reg_slots_pad[b] - reg_slots[b]) // P)
    sub_j = np.asarray(sub_j, np.int32)
    assert len(sub_j) * P == tot

    # chunk -> bank (for gather source AP)
    chunk_bank = []
    for b in range(nb):
        chunk_bank += [b] * int(reg_slots_pad[b] // CHUNK)
    chunk_bank = np.asarray(chunk_bank, np.int32)

    # slot position of every edge
    order = np.lexsort((tile, bank, core))
    gs = g[order]
    # occurrence rank within group (edges pre-sorted by group)
    grp_start = np.zeros(n_groups + 1, np.int64)
    np.cumsum(np.bincount(gs, minlength=n_groups), out=grp_start[1:])
    occ = np.arange(len(gs)) - grp_start[gs]
    slot = g_base[bank[order], tile[order]] + occ

    idx16 = np.zeros((C, tot), np.int16)
    dstl_a = np.full((C, tot), -1.0, np.float32)
    w_a = np.zeros((C, tot), np.float32)
    co = core[order]
    idx16[co, slot] = idx_in_bank[order]
    dstl_a[co, slot] = dstl[order]
    w_a[co, slot] = w[order]

    # device layouts
    # idx wrapped: [128, tot/16] (16-part blocks replicated x8)
    idx_w = np.zeros((C, 128, tot // 16), np.int16)
    meta = np.zeros((C, 128, (tot // P) * 2), np.float32)
    for c_ in range(C):
        blk = idx16[c_].reshape(-1, 16).T          # [16, tot/16]
        idx_w[c_] = np.tile(blk, (8, 1))
        d = dstl_a[c_].reshape(-1, P).T            # [128, tot/128]
        ww = w_a[c_].reshape(-1, P).T
        meta[c_, :, 0::2] = d
        meta[c_, :, 1::2] = ww
    return dict(idx=idx_w, meta=meta, sub_j=sub_j, chunk_bank=chunk_bank,
                nb=nb, bsz=bsz, tot=tot)


_CACHE = {}


def _build_and_compile(key, p1, p2, N, F, OUT, ND, NT, NDP, H3):
    from concourse import bass, bacc, mybir
    import concourse.tile as tile
    from concourse.masks import make_identity

    f32 = mybir.dt.float32
    i16 = mybir.dt.int16
    AF = mybir.ActivationFunctionType

    nc = bacc.Bacc("TRN2", target_bir_lowering=False, debug=False,
                   num_devices=C, num_swdge_queues=4)

    x_d = nc.dram_tensor("x", [N, F], f32, kind="ExternalInput")
    xblk_d = nc.dram_tensor("xblk", [NDP, F], f32, kind="ExternalInput")
    idx1_d = nc.dram_tensor("idx1", [128, p1["tot"] // 16], i16, kind="ExternalInput")
    meta1_d = nc.dram_tensor("meta1", [128, (p1["tot"] // P) * 2], f32, kind="ExternalInput")
    idx2_d = nc.dram_tensor("idx2", [128, p2["tot"] // 16], i16, kind="ExternalInput")
    meta2_d = nc.dram_tensor("meta2", [128, (p2["tot"] // P) * 2], f32, kind="ExternalInput")
    W0_d = nc.dram_tensor("W0", [F, F], f32, kind="ExternalInput")
    W1_d = nc.dram_tensor("W1", [F, F], f32, kind="ExternalInput")
    W2_d = nc.dram_tensor("W2", [F, F], f32, kind="ExternalInput")
    b0_d = nc.dram_tensor("b0", [F], f32, kind="ExternalInput")
    b1_d = nc.dram_tensor("b1", [F], f32, kind="ExternalInput")
    b2_d = nc.dram_tensor("b2", [F], f32, kind="ExternalInput")
    Wl_d = nc.dram_tensor("Wl", [H3, OUT], f32, kind="ExternalInput")
    bl_d = nc.dram_tensor("bl", [OUT], f32, kind="ExternalInput")
    out_d = nc.dram_tensor("out", [NDP, OUT], f32, kind="ExternalOutput")

    h1loc = nc.dram_tensor("h1loc", [NDP, F], f32)
    h1ag = nc.dram_tensor("h1ag", [NDP * C, F], f32, addr_space="Shared")

    qctr = [0]

    with tile.TileContext(nc) as tc:
        with tc.tile_pool(name="persist", bufs=1) as pp, \
             tc.tile_pool(name="sbuf", bufs=3) as pool, \
             tc.tile_pool(name="gpool", bufs=10) as gpool, \
             tc.tile_pool(name="mpool", bufs=10) as mpool, \
             tc.tile_pool(name="epool", bufs=6) as epool, \
             tc.tile_pool(name="psum_s", bufs=4, space="PSUM") as psum_s, \
             tc.tile_pool(name="psum_d", bufs=1, space="PSUM") as psum_d:

            ident = pp.tile([128, 128], f32)
            make_identity(nc, ident[:])
            iota_i = pp.tile([128, 128], mybir.dt.int32)
            nc.gpsimd.iota(iota_i[:], pattern=[[1, 128]], base=0, channel_multiplier=0)
            iota_f = pp.tile([128, 128], f32)
            nc.vector.tensor_copy(iota_f[:], iota_i[:])

            acc1 = pp.tile([128, NT * F], f32)
            acc2 = pp.tile([128, NT * F], f32)
            nc.vector.memset(acc1[:], 0.0)
            nc.vector.memset(acc2[:], 0.0)

            def propagate(prep, src_d, src_rows, acc):
                nb, bsz, tot = prep["nb"], prep["bsz"], prep["tot"]
                sub_j = prep["sub_j"]
                chunk_bank = prep["chunk_bank"]
                idx_d, meta_d = (idx1_d, meta1_d) if prep is p1 else (idx2_d, meta2_d)
                nchunks = tot // CHUNK
                for ch in range(nchunks):
                    b = int(chunk_bank[ch])
                    lo = b * bsz
                    hi = min(lo + bsz, src_rows)
                    idx_t = mpool.tile([128, CHUNK // 16], i16, tag="idx")
                    nc.sync.dma_start(out=idx_t[:], in_=idx_d[:, ch * (CHUNK // 16):(ch + 1) * (CHUNK // 16)])
                    meta_t = mpool.tile([128, CH_SUB * 2], f32, tag="meta")
                    nc.sync.dma_start(out=meta_t[:], in_=meta_d[:, ch * CH_SUB * 2:(ch + 1) * CH_SUB * 2])
                    g_t = gpool.tile([128, CH_SUB, F], f32, tag="g")
                    nc.gpsimd.dma_gather(
                        g_t[:], src_d[lo:hi, :], idx_t[:], CHUNK, CHUNK, F,
                        elem_step=F, queue_num=qctr[0] % 4)
                    qctr[0] += 1
                    for s in range(CH_SUB):
                        j = int(sub_j[ch * CH_SUB + s])
                        gs = g_t[:, s, :]
                        nc.vector.tensor_tensor(
                            out=gs, in0=gs,
                            in1=meta_t[:, 2 * s + 1:2 * s + 2].to_broadcast([128, F]),
                            op=mybir.AluOpType.mult)
                        eq = epool.tile([128, 128], f32, tag="eq")
                        nc.vector.tensor_tensor(
                            out=eq[:], in0=meta_t[:, 2 * s:2 * s + 1].to_broadcast([128, 128]),
                            in1=iota_f[:], op=mybir.AluOpType.is_equal)
                        ps = psum_s.tile([128, F], f32, space="PSUM", tag="pscat")
                        nc.tensor.matmul(out=ps[:], lhsT=eq[:], rhs=gs, start=True, stop=True)
                        nc.vector.tensor_add(out=acc[:, j * F:(j + 1) * F],
                                             in0=acc[:, j * F:(j + 1) * F], in1=ps[:])

            # ---- propagate 1: h1 = A_hat x ----
            propagate(p1, x_d, N, acc1)

            # evacuate h1 -> dram (tiled layout == row-major [NDP, F])
            nc.sync.dma_start(
                out=h1loc.rearrange("(j p) f -> p j f", p=128),
                in_=acc1[:].rearrange("p (j f) -> p j f", f=F))

            # ---- allgather h1 ----
            nc.gpsimd.collective_compute(
                "AllGather", mybir.AluOpType.bypass,
                replica_groups=[list(range(C))],
                ins=[h1loc[:]], outs=[h1ag[:]])

            # ---- propagate 2: h2 = A_hat h1 ----
            propagate(p2, h1ag, NDP * C, acc2)

            # ---- dense layers, per node tile ----
            W0_t = pp.tile([F, F], f32); nc.sync.dma_start(out=W0_t[:], in_=W0_d[:])
            W1_t = pp.tile([F, F], f32); nc.sync.dma_start(out=W1_t[:], in_=W1_d[:])
            W2_t = pp.tile([F, F], f32); nc.sync.dma_start(out=W2_t[:], in_=W2_d[:])
            b0_t = pp.tile([F, 1], f32); nc.sync.dma_start(out=b0_t[:], in_=b0_d[:, None])
            b1_t = pp.tile([F, 1], f32); nc.sync.dma_start(out=b1_t[:], in_=b1_d[:, None])
            b2_t = pp.tile([F, 1], f32); nc.sync.dma_start(out=b2_t[:], in_=b2_d[:, None])
            Wl1_t = pp.tile([128, OUT], f32); nc.sync.dma_start(out=Wl1_t[:], in_=Wl_d[0:128, :])
            Wl2_t = pp.tile([H3 - 128, OUT], f32); nc.sync.dma_start(out=Wl2_t[:], in_=Wl_d[128:H3, :])
            bl_t = pp.tile([OUT, 1], f32); nc.sync.dma_start(out=bl_t[:], in_=bl_d[:, None])

            # partition id -> x row offset of this core's block, via iota trick:
            # instead, x rows are loaded with the global offset baked per core.
            # SPMD: same program all cores -> use partition-id-dependent DMA?
            # Simpler: x block is replicated input; each core uses its own
            # node range. We pass the block rows via a per-core input tensor.
            for j in range(NT):
                xt_l = pool.tile([128, F], f32, tag="xtl")
                nc.sync.dma_start(out=xt_l[:], in_=xblk_d[j * 128:(j + 1) * 128, :])
                xT_ps = psum_d.tile([F, 128], f32, space="PSUM", tag="ptr")
                nc.tensor.transpose(out=xT_ps[:], in_=xt_l[:], identity=ident[:])
                xT = pool.tile([F, 128], f32, tag="xT")
                nc.vector.tensor_copy(xT[:], xT_ps[:])

                h1T_ps = psum_d.tile([F, 128], f32, space="PSUM", tag="ptr")
                nc.tensor.transpose(out=h1T_ps[:], in_=acc1[:, j * F:(j + 1) * F], identity=ident[:])
                h1T = pool.tile([F, 128], f32, tag="h1T")
                nc.vector.tensor_copy(h1T[:], h1T_ps[:])

                h2T_ps = psum_d.tile([F, 128], f32, space="PSUM", tag="ptr")
                nc.tensor.transpose(out=h2T_ps[:], in_=acc2[:, j * F:(j + 1) * F], identity=ident[:])
                h2T = pool.tile([F, 128], f32, tag="h2T")
                nc.vector.tensor_copy(h2T[:], h2T_ps[:])

                hT12 = pool.tile([128, 128], f32, tag="hT12")
                o_ps = psum_d.tile([F, 128], f32, space="PSUM", tag="pd")
                nc.tensor.matmul(out=o_ps[:], lhsT=W0_t[:], rhs=xT[:], start=True, stop=True)
                nc.scalar.activation(out=hT12[0:F, :], in_=o_ps[:], func=AF.Relu, bias=b0_t[:])
                o_ps2 = psum_d.tile([F, 128], f32, space="PSUM", tag="pd")
                nc.tensor.matmul(out=o_ps2[:], lhsT=W1_t[:], rhs=h1T[:], start=True, stop=True)
                nc.scalar.activation(out=hT12[F:2 * F, :], in_=o_ps2[:], func=AF.Relu, bias=b1_t[:])
                hT2 = pool.tile([H3 - 128, 128], f32, tag="hT2")
                o_ps3 = psum_d.tile([F, 128], f32, space="PSUM", tag="pd")
                nc.tensor.matmul(out=o_ps3[:], lhsT=W2_t[:], rhs=h2T[:], start=True, stop=True)
                nc.scalar.activation(out=hT2[:], in_=o_ps3[:], func=AF.Relu, bias=b2_t[:])

                of_ps = psum_d.tile([OUT, 128], f32, space="PSUM", tag="pf")
                nc.tensor.matmul(out=of_ps[:], lhsT=Wl1_t[:], rhs=hT12[:], start=True, stop=False)
                nc.tensor.matmul(out=of_ps[:], lhsT=Wl2_t[:], rhs=hT2[:], start=False, stop=True)
                oT = pool.tile([OUT, 128], f32, tag="oT")
                nc.scalar.activation(out=oT[:], in_=of_ps[:], func=AF.Identity, bias=bl_t[:])
                oo_ps = psum_d.tile([128, OUT], f32, space="PSUM", tag="po")
                nc.tensor.transpose(out=oo_ps[:], in_=oT[:], identity=ident[:OUT, :OUT])
                o_sb = pool.tile([128, OUT], f32, tag="osb")
                nc.vector.tensor_copy(o_sb[:], oo_ps[:])
                nc.sync.dma_start(out=out_d[j * 128:(j + 1) * 128, :], in_=o_sb[:])

    nc.compile()
    return nc


def kernel(x, edge_index, W0, b0, W1, b1, W2, b2, Wl, bl):
    from concourse.bass_utils import run_bass_kernel_spmd

    x = np.asarray(x, np.float32)
    ei = np.asarray(edge_index)
    N, F = x.shape
    E = ei.shape[1]
    OUT = Wl.shape[1]
    H3 = Wl.shape[0]
    ND = -(-N // C)
    NT = -(-ND // P)
    NDP = NT * P

    import hashlib
    key = (N, F, E, OUT, H3, hashlib.md5(np.ascontiguousarray(ei)).hexdigest())
    if key in _CACHE:
        nc, p1, p2 = _CACHE[key]
        return _run(nc, p1, p2, x, W0, b0, W1, b1, W2, b2, Wl, bl, N, F, ND, NDP)

    src = ei[0].astype(np.int64)
    dst = ei[1].astype(np.int64)
    deg = np.bincount(dst, minlength=N) + 1.0
    dinv = (1.0 / np.sqrt(deg)).astype(np.float64)
    sa = np.concatenate([src, np.arange(N, dtype=np.int64)])
    da = np.concatenate([dst, np.arange(N, dtype=np.int64)])
    w = (dinv[sa] * dinv[da]).astype(np.float32)

    p1 = _prep_edges(sa, da, w, N, N, ND, NT, C)
    # P2 source rows live in the padded/tiled h1 space: row = c*NDP + (n - c*ND)
    core_s = sa // ND
    sa2 = core_s * NDP + (sa - core_s * ND)
    p2 = _prep_edges(sa2, da, w, NDP * C, N, ND, NT, C)

    nc = _build_and_compile(None, p1, p2, N, F, OUT, ND, NT, NDP, H3)
    _CACHE[key] = (nc, p1, p2)
    return _run(nc, p1, p2, x, W0, b0, W1, b1, W2, b2, Wl, bl, N, F, ND, NDP)


def _run(nc, p1, p2, x, W0, b0, W1, b1, W2, b2, Wl, bl, N, F, ND, NDP):
    from concourse.bass_utils import run_bass_kernel_spmd

    ins = []
    for c in range(C):
        xblk = np.zeros((NDP, F), np.float32)
        lo = c * ND
        hi = min(lo + NDP, N)
        if hi > lo:
            xblk[:hi - lo] = x[lo:hi]
        ins.append({
            "x": x,
            "xblk": xblk,
            "idx1": p1["idx"][c], "meta1": p1["meta"][c],
            "idx2": p2["idx"][c], "meta2": p2["meta"][c],
            "W0": np.asarray(W0, np.float32), "W1": np.asarray(W1, np.float32),
            "W2": np.asarray(W2, np.float32),
            "b0": np.asarray(b0, np.float32), "b1": np.asarray(b1, np.float32),
            "b2": np.asarray(b2, np.float32),
            "Wl": np.asarray(Wl, np.float32), "bl": np.asarray(bl, np.float32),
        })
    res = run_bass_kernel_spmd(nc, ins, list(range(C)))
    out = np.concatenate([res.results[c]["out"][:min(ND, N - c * ND)] for c in range(C)], 0)
    return out.astype(np.float32)
